# revision 2
# baseline (speedup 1.0000x reference)
"""BitLinear (BitNet b1.58) forward kernel for Trainium2, 8 NeuronCores.

y = act_quant(x) @ weight_quant(W)^T + bias
  - activation quant: per-token absmax int8 fake-quant (values in [-127,127])
  - weight quant: per-tensor mean-absmax ternary fake-quant {-1,0,1}

Sharding: data-parallel over the batch dim (8 batches -> 1 per core);
W and bias are replicated per core, each core computes mean(|W|) locally
(no collectives needed).

Numerics (rel err ~2.5e-3 vs fp32 reference, gate is 2e-2):
  * q = round(x * s) with s = 127*recip(mx): exact int8 values in bf16; the
    ternary t in {-1,0,1} is exact in bf16; the PE accumulates exact integer
    sums (< 2^24) in fp32 PSUM. Epilogue multiplies by c_tok = mx*mean|W|/127
    and adds fp32 bias, stores y in bf16 (the only deliberate precision loss).
  * mean(|W|) must match the reference's fp32 value to ~2e-7 relative (nearest
    weight sits 2.6e-7 from a ternary rounding boundary), so the reduction
    uses an exact hi/lo split summation.

Engine layout per token tile: PE does ONLY the 16 N=512 bf16 matmuls (no
transposes, no bias matmuls). All transposes (W chunks and the per-tile q)
run on the DMA xbar tile-transpose engine. ACT does the x-scale pass and the
PSUM*c_tok epilogue; DVE does the quant chain and the per-column bias add
with the bf16 downcast. y is stored bf16 and upcast on the host.
"""

import os
import sys

import numpy as np

B, S, DIN, DOUT = 8, 4096, 512, 2048
N_CORES = 8

MAGIC = 12582912.0  # 1.5 * 2^23: (v + MAGIC) - MAGIC == round-half-even(v), |v| < 2^22
C_GRID_11 = 6144.0  # 1.5 * 2^12: rounds to multiples of 2^-11 (values <= ~26)
C_GRID_4 = 786432.0  # 1.5 * 2^19: rounds to multiples of 2^-4  (values <= ~400)
EPS = 1e-6

_cached = {}


def _ensure_path():
    try:
        import concourse  # noqa: F401
    except ImportError:
        for p in ("/opt/trn_rl_repo", os.path.expanduser("~/.axon_site/_ro/trn_rl_repo")):
            if os.path.isdir(p) and p not in sys.path:
                sys.path.insert(0, p)


def build_program(s_tiles=S // 128):
    """Emit the Bass/Tile program for one core: x [s_tiles*128, DIN] -> y."""
    _ensure_path()
    from contextlib import ExitStack

    import concourse.bacc as bacc
    import concourse.tile as tile
    from concourse import mybir

    f32 = mybir.dt.float32
    bf16 = mybir.dt.bfloat16
    Alu = mybir.AluOpType
    X = mybir.AxisListType.X
    Copy = mybir.ActivationFunctionType.Copy
    SROWS = s_tiles * 128

    nc = bacc.Bacc("TRN2", target_bir_lowering=False, debug=False, num_devices=N_CORES)
    x_d = nc.dram_tensor("x", [SROWS, DIN], f32, kind="ExternalInput").ap()
    w_d = nc.dram_tensor("w", [DOUT, DIN], f32, kind="ExternalInput").ap()
    b_d = nc.dram_tensor("bias", [1, DOUT], f32, kind="ExternalInput").ap()
    y_d = nc.dram_tensor("y", [SROWS, DOUT], bf16, kind="ExternalOutput").ap()

    KC = DIN // 128  # 4 contraction chunks
    OC = DOUT // 128  # 16 output chunks

    with tile.TileContext(nc) as tc, ExitStack() as ctx:
        cpool = ctx.enter_context(tc.tile_pool(name="const", bufs=1))
        wallp = ctx.enter_context(tc.tile_pool(name="wall", bufs=1))
        wtmpp = ctx.enter_context(tc.tile_pool(name="wtmp", bufs=3))
        wqp = ctx.enter_context(tc.tile_pool(name="wq", bufs=3))
        statp = ctx.enter_context(tc.tile_pool(name="stat", bufs=1))
        tTp = ctx.enter_context(tc.tile_pool(name="tT", bufs=1))
        xp = ctx.enter_context(tc.tile_pool(name="x", bufs=6))
        r1p = ctx.enter_context(tc.tile_pool(name="r1", bufs=4))
        qp = ctx.enter_context(tc.tile_pool(name="q", bufs=4))
        qtp = ctx.enter_context(tc.tile_pool(name="qt", bufs=6))
        mxp = ctx.enter_context(tc.tile_pool(name="mx", bufs=16))
        yfp = ctx.enter_context(tc.tile_pool(name="yf", bufs=3))
        yp = ctx.enter_context(tc.tile_pool(name="y", bufs=3))
        pstat = ctx.enter_context(tc.tile_pool(name="pstat", bufs=1, space="PSUM"))
        py = ctx.enter_context(tc.tile_pool(name="py", bufs=3, space="PSUM"))

        # ---- W load first: per-chunk DMAs so abs-sums start immediately, and
        # nothing else sits ahead of them in the sync HWDGE FIFO ----
        w_all = wallp.tile([128, OC, DIN], f32)
        w_r = w_d.rearrange("(c p) d -> p c d", p=128)
        for c in range(OC):
            nc.sync.dma_start(w_all[:, c : c + 1, :], w_r[:, c : c + 1, :])

        # ---- constants ----
        b_row = cpool.tile([1, DOUT], f32)
        nc.scalar.dma_start(b_row[:], b_d)
        bias_bc = cpool.tile([128, DOUT], f32)
        nc.gpsimd.dma_start(bias_bc[:], b_d.broadcast_to([128, DOUT]))
        ones128 = cpool.tile([128, 128], f32)
        nc.vector.memset(ones128[:], 1.0)

        # ---- mean(|W|): exact-split summation ----
        wsum = statp.tile([128, OC], f32)
        for c in range(OC):
            nc.vector.tensor_reduce(
                wsum[:, c : c + 1], w_all[:, c, :],
                axis=X, op=Alu.add, apply_absolute_value=True,
            )
        # split per-chunk sums (<= ~26) to a 2^-11 grid -> exact 16-way add
        hh = statp.tile([128, OC], f32)
        ll = statp.tile([128, OC], f32)
        nc.vector.tensor_scalar(hh[:], wsum[:], C_GRID_11, C_GRID_11, op0=Alu.add, op1=Alu.subtract)
        nc.vector.tensor_tensor(ll[:], wsum[:], hh[:], op=Alu.subtract)
        hs = statp.tile([128, 1], f32)
        ls = statp.tile([128, 1], f32)
        nc.vector.tensor_reduce(hs[:], hh[:], axis=X, op=Alu.add)
        nc.vector.tensor_reduce(ls[:], ll[:], axis=X, op=Alu.add)
        # split per-partition totals (<= ~400) to a 2^-4 grid -> exact 128-way add
        red = statp.tile([128, 2], f32)
        l2 = statp.tile([128, 1], f32)
        nc.vector.tensor_scalar(red[:, 0:1], hs[:], C_GRID_4, C_GRID_4, op0=Alu.add, op1=Alu.subtract)
        nc.vector.tensor_tensor(l2[:], hs[:], red[:, 0:1], op=Alu.subtract)
        nc.vector.tensor_tensor(red[:, 1:2], l2[:], ls[:], op=Alu.add)
        # cross-partition sum + broadcast in one exact fp32 ones-matmul:
        # out[m, j] = sum_p red[p, j] for every m
        pred = pstat.tile([128, 2], f32, tag="pred", name="pred")
        nc.tensor.matmul(pred[:], ones128[:], red[:], start=True, stop=True)
        redo = statp.tile([128, 2], f32)
        nc.scalar.copy(redo[:], pred[:])
        ssum = statp.tile([128, 1], f32)
        nc.vector.tensor_tensor(ssum[:], redo[:, 0:1], redo[:, 1:2], op=Alu.add)
        mean_t = statp.tile([128, 1], f32)
        nc.vector.tensor_scalar(mean_t[:], ssum[:], 1.0 / (DOUT * DIN), None, op0=Alu.mult)
        nc.vector.tensor_scalar(mean_t[:], mean_t[:], EPS, None, op0=Alu.max)
        s_w = statp.tile([128, 1], f32)  # 1/mean: the quantization scale
        nc.vector.reciprocal(s_w[:], mean_t[:])
        v_w = statp.tile([128, 1], f32)  # fl(1/s_w): the dequant magnitude (matches ref)
        nc.vector.reciprocal(v_w[:], s_w[:])
        vw127 = statp.tile([128, 1], f32)  # v_w / 127, folded once for the epilogue scale
        nc.vector.tensor_scalar(vw127[:], v_w[:], 1.0 / 127.0, None, op0=Alu.mult)

        # ---- W quantize; transpose on the DMA xbar: tT[:, k, o] = t[o, k*128+p] ----
        tT = tTp.tile([128, KC, DOUT], bf16)
        for c in range(OC):
            wc = w_all[:, c, :]
            wr1 = wtmpp.tile([128, DIN], f32, tag="wr1")
            nc.scalar.activation(wr1[:], wc, Copy, bias=MAGIC, scale=s_w[:])
            wr2 = wtmpp.tile([128, DIN], f32, tag="wr2")
            nc.vector.tensor_scalar(wr2[:], wr1[:], MAGIC, 1.0, op0=Alu.subtract, op1=Alu.min)
            wq = wqp.tile([128, DIN], bf16)
            nc.vector.tensor_scalar(wq[:], wr2[:], -1.0, None, op0=Alu.max)
            nc.sync.dma_start_transpose(tT[:, :, c * 128 : (c + 1) * 128], wq[:])

        # ---- main loop over token tiles ----
        for i in range(s_tiles):
            xt = xp.tile([128, DIN], f32)
            nc.scalar.dma_start(xt[:], x_d[i * 128 : (i + 1) * 128, :])

            mx = mxp.tile([128, 1], f32, tag="mx")
            nc.vector.tensor_reduce(mx[:], xt[:], axis=X, op=Alu.max, apply_absolute_value=True)
            sx = mxp.tile([128, 1], f32, tag="sx")
            nc.vector.reciprocal(sx[:], mx[:])
            nc.vector.tensor_scalar(sx[:], sx[:], 127.0, None, op0=Alu.mult)
            c_tok = mxp.tile([128, 1], f32, tag="ct")
            nc.vector.tensor_tensor(c_tok[:], mx[:], vw127[:], op=Alu.mult)

            r1 = r1p.tile([128, DIN], f32)
            nc.scalar.activation(r1[:], xt[:], Copy, bias=MAGIC, scale=sx[:])
            q = qp.tile([128, DIN], bf16)
            nc.vector.tensor_scalar(q[:], r1[:], MAGIC, None, op0=Alu.subtract)

            qT = qtp.tile([128, KC, 128], bf16)
            nc.sync.dma_start_transpose(qT[:], q[:])

            yf = yfp.tile([128, DOUT], f32)
            ysb = yp.tile([128, DOUT], bf16)
            for h in range(2):
                ph = py.tile([128, 1024], f32, tag="ytile", name="ph")
                base = h * 1024
                for k in range(KC):
                    lhsT = qT[:, k, :]
                    for n in range(2):
                        nc.tensor.matmul(
                            ph[:, n * 512 : (n + 1) * 512], lhsT,
                            tT[:, k, base + n * 512 : base + (n + 1) * 512],
                            start=(k == 0), stop=(k == KC - 1),
                        )
                sl = slice(base, base + 1024)
                nc.scalar.activation(yf[:, sl], ph[:], Copy, scale=c_tok[:])
                nc.vector.tensor_tensor(ysb[:, sl], yf[:, sl], bias_bc[:, sl], op=Alu.add)
            nc.gpsimd.dma_start(y_d[i * 128 : (i + 1) * 128, :], ysb[:])

    nc.compile()
    return nc


def _get_program():
    if "nc" not in _cached:
        _cached["nc"] = build_program()
    return _cached["nc"]


def kernel(x: np.ndarray, weight: np.ndarray, bias: np.ndarray) -> np.ndarray:
    _ensure_path()
    from concourse.bass_utils import run_bass_kernel_spmd

    x = np.ascontiguousarray(x, dtype=np.float32)
    weight = np.ascontiguousarray(weight, dtype=np.float32)
    bias2d = np.ascontiguousarray(bias, dtype=np.float32).reshape(1, DOUT)

    nc = _get_program()
    in_maps = [
        {"x": x[c], "w": weight, "bias": bias2d} for c in range(N_CORES)
    ]
    res = run_bass_kernel_spmd(nc, in_maps, core_ids=list(range(N_CORES)))
    _cached["last_results"] = res
    y = np.stack(
        [np.asarray(res.results[c]["y"]).astype(np.float32) for c in range(N_CORES)],
        axis=0,
    )
    return y
